# revision 4
# baseline (speedup 1.0000x reference)
"""Trainium2 Bass kernel for nn_Block_70944269795510 (involution block).

Strategy (8 NeuronCores, data-parallel over batch: 2 samples/core):
  conv1 (PE fp32) -> tanh/bn1 (ACT, evac) -> y bf16 (dense + padded copies)
  red  (PE bf16)  -> relu/bn  (ACT)       -> r bf16
  span (PE bf16, weights pre-replicated x16 across group channels on host)
       -> per-pixel kernels wd_b land in PSUM already broadcast over the 16
          channels of each group; ACT evacuates to SBUF bf16 (+span_b bias)
  involution: 49 shift-mult-accumulate passes on the DVE (bf16 2x mode),
       shifts are pure access-pattern offsets into padded-Y (two parity
       copies keep bf16 reads 4B-aligned for any kernel-column offset)
  tanh/bn2 (ACT) -> conv3 (PE bf16) -> bn3 (ACT) -> +x (DVE) -> DMA out
"""

import sys

for _p in ("/opt/trn_rl_repo", "/root/.axon_site/_ro/trn_rl_repo"):
    if _p not in sys.path:
        sys.path.append(_p)

import numpy as np
import ml_dtypes
from contextlib import ExitStack

import concourse.bass as bass
import concourse.mybir as mybir
from concourse import bacc
from concourse.tile import TileContext
from concourse.bass_utils import run_bass_kernel_spmd

F32 = mybir.dt.float32
BF16 = mybir.dt.bfloat16
AF = mybir.ActivationFunctionType

B, C, H, W = 16, 256, 56, 56
HW = H * W                      # 3136
KK = 7                          # involution kernel size
G, GC, RED = 16, 16, 64
EPS = 1e-5
PAD = 3
N_CORES = 8
S_PER_CORE = B // N_CORES       # 2 samples per core
PW = 64                         # padded row stride (W + 2*PAD rounded up to 64)
HP = H + 2 * PAD                # 62 padded rows
NPAD = HP * PW                  # 3968 padded pixels per partition

# hw tiling: 7 row-blocks of 8 rows (448 px); PSUM halves of 1568 px
T448 = 448
HALF = 1568


def _build_nc():
    nc = bacc.Bacc("TRN2", target_bir_lowering=False, debug=False)

    xd = nc.dram_tensor("x", [S_PER_CORE, 2, 128, HW], F32, kind="ExternalInput").ap()
    w1d = nc.dram_tensor("w1t", [128, 2, 256], F32, kind="ExternalInput").ap()
    rwd = nc.dram_tensor("rwt", [128, 2, 64], BF16, kind="ExternalInput").ap()
    srd = nc.dram_tensor("srep", [64, 2, 49 * 128], BF16, kind="ExternalInput").ap()
    brd = nc.dram_tensor("brep", [128, 2, 49], F32, kind="ExternalInput").ap()
    w3d = nc.dram_tensor("w3t", [128, 2, 256], BF16, kind="ExternalInput").ap()
    scd = nc.dram_tensor("scal", [128, 2, 6], F32, kind="ExternalInput").ap()
    srcd = nc.dram_tensor("scred", [64, 2], F32, kind="ExternalInput").ap()
    outd = nc.dram_tensor("out", [S_PER_CORE, 2, 128, HW], F32, kind="ExternalOutput").ap()

    with TileContext(nc) as tc, ExitStack() as ctx:
        consts = ctx.enter_context(tc.tile_pool(name="consts", bufs=1))
        xpool = ctx.enter_context(tc.tile_pool(name="xp", bufs=2))
        ydpool = ctx.enter_context(tc.tile_pool(name="ydp", bufs=2))
        ypapool = ctx.enter_context(tc.tile_pool(name="ypa", bufs=2))
        ypbpool = ctx.enter_context(tc.tile_pool(name="ypb", bufs=2))
        rpool = ctx.enter_context(tc.tile_pool(name="rp", bufs=2))
        wdpool = ctx.enter_context(tc.tile_pool(name="wdp", bufs=3))
        prpool = ctx.enter_context(tc.tile_pool(name="prp", bufs=2))
        accpool = ctx.enter_context(tc.tile_pool(name="accp", bufs=2))
        y2pool = ctx.enter_context(tc.tile_pool(name="y2p", bufs=2))
        opool = ctx.enter_context(tc.tile_pool(name="op", bufs=1))
        pspool = ctx.enter_context(tc.tile_pool(name="psp", bufs=2, space="PSUM"))

        # constants
        w1t = consts.tile([128, 2, 256], F32)
        nc.sync.dma_start(out=w1t, in_=w1d)
        rwt = consts.tile([128, 2, 64], BF16)
        nc.sync.dma_start(out=rwt, in_=rwd)
        srep = consts.tile([64, 2, 49 * 128], BF16)
        nc.sync.dma_start(out=srep, in_=srd)
        brep = consts.tile([128, 2, 49], F32)
        nc.sync.dma_start(out=brep, in_=brd)
        w3t = consts.tile([128, 2, 256], BF16)
        nc.sync.dma_start(out=w3t, in_=w3d)
        scal = consts.tile([128, 2, 6], F32)
        nc.sync.dma_start(out=scal, in_=scd)
        scred = consts.tile([64, 2], F32)
        nc.sync.dma_start(out=srcd_sb_fix(scred), in_=srcd)

        # 7 row-block tiles grouped (4, 3) per 4-bank PSUM tile
        groups = [(0, 4), (4, 3)]

        for s in range(S_PER_CORE):
            xs = []
            for m in range(2):
                xt = xpool.tile([128, HW], F32, tag="x")
                nc.sync.dma_start(out=xt, in_=xd[s, m])
                xs.append(xt)

            # ---- conv1 + tanh/bn1 -> y dense bf16 + padded copies ----
            yd, ypa, ypb = [], [], []
            for m in range(2):
                ydt = ydpool.tile([128, HW], BF16, tag="yd")
                ydv = ydt[:].rearrange("p (t w) -> p t w", w=T448)
                for g0, cnt in groups:
                    ps = pspool.tile([128, 2048], F32, tag="ps")
                    psv = ps[:].rearrange("p (t w) -> p t w", w=512)
                    for ti in range(cnt):
                        t = g0 + ti
                        for k in range(2):
                            nc.tensor.matmul(
                                psv[:, ti, 0:T448],
                                w1t[:, k, m * 128:(m + 1) * 128],
                                xs[k][:, t * T448:(t + 1) * T448],
                                start=(k == 0), stop=(k == 1),
                            )
                    nc.scalar.activation(
                        out=ydv[:, g0:g0 + cnt, :],
                        in_=psv[:, 0:cnt, 0:T448],
                        func=AF.Tanh,
                        scale=scal[:, m, 0:1],
                        bias=scal[:, m, 1:2],
                    )
                yat = ypapool.tile([128, NPAD], BF16, tag="ypa")
                ybt = ypbpool.tile([128, NPAD], BF16, tag="ypb")
                nc.gpsimd.memset(yat[:], 0.0)
                yav = yat[:].rearrange("p (h w) -> p h w", w=PW)
                nc.gpsimd.tensor_copy(
                    out=yav[:, PAD:PAD + H, PAD:PAD + W],
                    in_=ydt[:].rearrange("p (h w) -> p h w", w=W),
                )
                nc.gpsimd.tensor_copy(out=ybt[:, 0:NPAD - 1], in_=yat[:, 1:NPAD])
                yd.append(ydt)
                ypa.append(yat)
                ypb.append(ybt)

            # ---- red conv + relu/bn -> r bf16 [64, HW] ----
            rt = rpool.tile([64, HW], BF16, tag="r")
            rv = rt[:].rearrange("p (t w) -> p t w", w=T448)
            for g0, cnt in groups:
                ps = pspool.tile([64, 2048], F32, tag="ps")
                psv = ps[:].rearrange("p (t w) -> p t w", w=512)
                for ti in range(cnt):
                    t = g0 + ti
                    for k in range(2):
                        nc.tensor.matmul(
                            psv[:, ti, 0:T448],
                            rwt[:, k, :],
                            yd[k][:, t * T448:(t + 1) * T448],
                            start=(k == 0), stop=(k == 1),
                        )
                nc.scalar.activation(
                    out=rv[:, g0:g0 + cnt, :],
                    in_=psv[:, 0:cnt, 0:T448],
                    func=AF.Relu,
                    scale=scred[:, 0:1],
                    bias=scred[:, 1:2],
                )

            # ---- involution per chunk: span matmul -> ACT evac -> DVE MAC ----
            y2s = []
            for m in range(2):
                acc = accpool.tile([128, HW], BF16, tag="acc")
                yav = ypa[m][:].rearrange("p (h w) -> p h w", w=PW)
                ybv = ypb[m][:].rearrange("p (h w) -> p h w", w=PW)
                for p in range(49):
                    wd = wdpool.tile([128, HW], BF16, tag="wd")
                    for half in range(2):
                        base = half * HALF
                        ps = pspool.tile([128, HALF], F32, tag="ps")
                        for off, n in ((0, 512), (512, 512), (1024, 512), (1536, 32)):
                            nc.tensor.matmul(
                                ps[:, off:off + n],
                                srep[:, m, p * 128:(p + 1) * 128],
                                rt[:, base + off:base + off + n],
                                start=True, stop=True,
                            )
                        nc.scalar.activation(
                            out=wd[:, base:base + HALF],
                            in_=ps[:, 0:HALF],
                            func=AF.Identity,
                            scale=1.0,
                            bias=brep[:, m, p:p + 1],
                        )
                    di, dj = p // KK, p % KK
                    if dj % 2 == 0:
                        ysh = yav[:, di:di + H, dj:dj + W]
                    else:
                        ysh = ybv[:, di:di + H, dj - 1:dj - 1 + W]
                    wd3 = wd[:].rearrange("p (h w) -> p h w", w=W)
                    if p == 0:
                        nc.vector.tensor_mul(
                            acc[:].rearrange("p (h w) -> p h w", w=W), wd3, ysh
                        )
                    else:
                        pr = prpool.tile([128, HW], BF16, tag="pr")
                        nc.vector.tensor_mul(
                            pr[:].rearrange("p (h w) -> p h w", w=W), wd3, ysh
                        )
                        nc.vector.tensor_add(acc[:], acc[:], pr[:])

                y2 = y2pool.tile([128, HW], BF16, tag="y2")
                nc.scalar.activation(
                    out=y2[:], in_=acc[:], func=AF.Tanh,
                    scale=scal[:, m, 2:3], bias=scal[:, m, 3:4],
                )
                y2s.append(y2)

            # ---- conv3 + bn3 + skip ----
            for m in range(2):
                ot = opool.tile([128, HW], F32, tag="o")
                ov = ot[:].rearrange("p (t w) -> p t w", w=T448)
                for g0, cnt in groups:
                    ps = pspool.tile([128, 2048], F32, tag="ps")
                    psv = ps[:].rearrange("p (t w) -> p t w", w=512)
                    for ti in range(cnt):
                        t = g0 + ti
                        for k in range(2):
                            nc.tensor.matmul(
                                psv[:, ti, 0:T448],
                                w3t[:, k, m * 128:(m + 1) * 128],
                                y2s[k][:, t * T448:(t + 1) * T448],
                                start=(k == 0), stop=(k == 1),
                            )
                    nc.scalar.activation(
                        out=ov[:, g0:g0 + cnt, :],
                        in_=psv[:, 0:cnt, 0:T448],
                        func=AF.Identity,
                        scale=scal[:, m, 4:5],
                        bias=scal[:, m, 5:6],
                    )
                nc.vector.tensor_add(ot[:], ot[:], xs[m][:])
                nc.sync.dma_start(out=outd[s, m], in_=ot[:])

    nc.compile()
    return nc


def srcd_sb_fix(t):
    # scred tile is [64, 2]; DMA full
    return t


def _bn_fold(g, b, m, v):
    s = (g / np.sqrt(v + EPS)).astype(np.float32)
    return s, (b - m * s).astype(np.float32)


def _prep_inputs(inputs):
    """Host-side preprocessing into SBUF-ready layouts (numpy, cheap)."""
    bf = ml_dtypes.bfloat16
    f32 = np.float32

    s1, t1 = _bn_fold(inputs["bn1_g"], inputs["bn1_b"], inputs["bn1_m"], inputs["bn1_v"])
    t1 = t1 + s1 * inputs["b1"]
    sr, tr = _bn_fold(inputs["red_bn_g"], inputs["red_bn_b"], inputs["red_bn_m"], inputs["red_bn_v"])
    tr = tr + sr * inputs["red_b"]
    s2, t2 = _bn_fold(inputs["bn2_g"], inputs["bn2_b"], inputs["bn2_m"], inputs["bn2_v"])
    s3, t3 = _bn_fold(inputs["bn3_g"], inputs["bn3_b"], inputs["bn3_m"], inputs["bn3_v"])
    t3 = t3 + s3 * inputs["b3"]

    w1t = np.ascontiguousarray(
        inputs["w1"].T.reshape(2, 128, 256).transpose(1, 0, 2)
    ).astype(f32)
    rwt = np.ascontiguousarray(
        inputs["red_w"].T.reshape(2, 128, 64).transpose(1, 0, 2)
    ).astype(bf)
    w3t = np.ascontiguousarray(
        inputs["w3"].T.reshape(2, 128, 256).transpose(1, 0, 2)
    ).astype(bf)

    # span weights replicated over the 16 channels of each group:
    # srep[e, m, p*128 + 16*g' + c] = span_w[(8m+g')*49 + p, e]
    sw = inputs["span_w"].reshape(G, 49, RED)            # [g, p, e]
    t = sw.transpose(2, 0, 1).reshape(RED, 2, 8, 49)     # [e, m, g', p]
    srep = np.repeat(t[:, :, :, None, :], GC, axis=3)    # [e, m, g', c, p]
    srep = np.ascontiguousarray(
        srep.transpose(0, 1, 4, 2, 3).reshape(RED, 2, 49 * 128)
    ).astype(bf)

    sb = inputs["span_b"].reshape(G, 49).reshape(2, 8, 49)
    brep = np.repeat(sb[:, :, None, :], GC, axis=2).reshape(2, 128, 49)
    brep = np.ascontiguousarray(brep.transpose(1, 0, 2)).astype(f32)

    scal = np.stack([s1, t1, s2, t2, s3, t3], axis=-1)   # [256, 6]
    scal = np.ascontiguousarray(scal.reshape(2, 128, 6).transpose(1, 0, 2)).astype(f32)
    scred = np.stack([sr, tr], axis=-1).astype(f32)      # [64, 2]

    x = inputs["x"].reshape(B, 2, 128, HW).astype(f32)   # [b, chunk, part, hw]

    common = dict(w1t=w1t, rwt=rwt, srep=srep, brep=brep, w3t=w3t, scal=scal, scred=scred)
    in_maps = []
    for i in range(N_CORES):
        shard = np.ascontiguousarray(x[i * S_PER_CORE:(i + 1) * S_PER_CORE])
        in_maps.append({**common, "x": shard})
    return in_maps


_NC = None


def _get_nc():
    global _NC
    if _NC is None:
        _NC = _build_nc()
    return _NC


def kernel(**inputs):
    inputs = {k: np.asarray(v) for k, v in inputs.items()}
    nc = _get_nc()
    in_maps = _prep_inputs(inputs)
    res = run_bass_kernel_spmd(nc, in_maps, list(range(N_CORES)))
    outs = [res.results[i]["out"].reshape(S_PER_CORE, C, H, W) for i in range(N_CORES)]
    return np.concatenate(outs, axis=0).astype(np.float32)
